# revision 50
# baseline (speedup 1.0000x reference)
"""Trainium2 Bass kernel for the depth-2 TT-compressed meta-linear module.

Math (per token t, with x the (D,)-vector of that token, repeated DEPTH=2):
    w0[r]      = sum_d x[d] * core0[0,d,r]
    y1[r,R]    = sum_d x[d] * core1[r,d,R]
    w1[R]      = sum_r w0[r] * y1[r,R]
    y2[r,R]    = sum_d x[d] * core2[r,d,R]
    w2[R]      = sum_r w1[r] * y2[r,R]
    x'[d]      = sum_R w2[R] * core3[R,d,0]
Output = x'' + bias.

Device mapping (8-way data parallel over tokens; 2048 tokens/core; four
512-token blocks per core, software-pipelined):
  - x is pre-split on the host into an fp8 hi/lo pair in a d-major layout
    [128, 2, NCH, T_CORE] (partition = d%128, chunk = d//128), one DMA per
    block.  The x-contractions run as fp8 DoubleRow matmuls (0.5
    cycles/row, K=256 per pass) using the 3-term expansion
    x*c ~ xh*ch + xh*cl + xl*ch (the xl*cl term is ~0.1% and dropped).
    Weights are scaled by WS=8192 to sit in fp8 range; 1/WS^3 is folded
    into the depth-boundary constants so depth 2 is unscaled.  Measured
    end-to-end error 2.4e-3 vs the 2e-2 gate.
  - Pass packing per depth: pass B computes [w0rep | y2] into one PSUM
    bank, pass A computes y1 into the bottom half of a second bank; the
    SREP r-sum matmul later writes pw into that bank's top half (a matmul
    with a partition-offset output, tile_position=(0,64) -- requires
    16-bit operands, hence the bf16 srep constant and z1 tiles; walrus
    rejects f32r there).
  - ONE Act copy stages the whole [w0rep | y2] bank to SBUF (engine cost
    is free-size-based, so staging both operands in one op costs the same
    as staging either).  Each z-chain multiply then reads one PSUM operand
    and one SBUF operand (walrus rejects two PSUM reads in one op), and
    stage 1 sits OFF the critical z-chain (hidden under pass A):
        z1 = y1[PSUM] * w0s[SBUF];  pw = SREP.T z1;  z2 = pw[PSUM]*y2s[SBUF]
  - The depth boundary is linear and host-folded: MB = C3S@[C0rep|C2],
    MA = C3S@C1 map z2 straight to depth-2's [w0rep|y2] and y1.
  - Final: out_tile = z2d2_chunk^T @ (S2@C3) emits token-major [t, d]
    tiles; PSUM->SBUF copies (6 Act / 2 DVE per block) downconvert to bf16
    (bf16 keeps fp32's exponent range; outputs are ~1e-15 so fp16 would
    flush) and halve output DMA traffic.  The last block's z2b multiply is
    emitted in token-tile thirds so its finals/copies/drain start on the
    first piece.
  - PSUM: 2 banks ping-pong for the A-passes, 2 for the B-passes, 4 for
    the finals ring.  A PE warm-up chain rides out the p-state ramp while
    the first DMAs stream in.
  - Steady state is jointly PE-bound (~5.1us/block: 24 fp8-DR passes, 4
    small matmuls, 8 finals) and DMA-bound (~5.8us/block in+out at
    360GB/s), with Act (stages+copies) and DVE (4 z-muls + copies) just
    below; the ends are DMA-latency (start) and drain (last block's
    copies + 4x728ns out transfers + sem prop) bound.
"""

import numpy as np

import concourse.bacc as bacc
import concourse.tile as tile
import concourse.mybir as mybir
import concourse.bass_utils as bass_utils

import os

N_CORES = 8
B, N, D, R = 4, 4096, 1024, 8
T_TOTAL = B * N              # 16384 tokens
T_CORE = T_TOTAL // N_CORES  # 2048 tokens per core
NCH = D // 128               # 8 d-chunks
TTILES = T_CORE // 128       # 16 token-tiles per core

BS = [int(v) for v in os.environ.get("K_BS", "512,512,512,512").split(",")]
assert sum(BS) == T_CORE
NBLK = len(BS)

F32R = mybir.dt.float32r
F32 = mybir.dt.float32
F16 = mybir.dt.float16
BF16 = mybir.dt.bfloat16
F8 = mybir.dt.float8e4

# fp8 weight scale: c0/c1/c2 entries are ~1e-3, below fp8's subnormal floor,
# so weights are stored as fp8(c * WS); the depth-boundary constants fold
# 1/WS^3 so everything downstream of z2 is unscaled.
WS = 8192.0

OUTSPLIT = int(os.environ.get("K_OUTSPLIT", "4"))
WARM = int(os.environ.get("K_WARM", "6"))
DR = mybir.MatmulPerfMode.DoubleRow


def _build_program(with_bias=False):
    nc = bacc.Bacc("TRN2", target_bir_lowering=False, debug=False,
                   num_devices=N_CORES)

    zp = 65 if with_bias else 64  # final contraction size (65 = +bias row)

    # hi/lo interleaved on dim 1 so one DMA per block loads both
    x_d = nc.dram_tensor("x8", [128, 2, NCH, T_CORE], F8,
                         kind="ExternalInput")
    out_d = nc.dram_tensor("out", [128, TTILES, D], BF16,
                           kind="ExternalOutput")
    # cbh | cbl | cah | cal packed along the last dim: one DMA
    cp_d = nc.dram_tensor("cp", [128, NCH, 384], F8, kind="ExternalInput")
    # MB | MA packed along the free dim
    sm_d = nc.dram_tensor("sm", [64, 192], F32R, kind="ExternalInput")
    # srep is bf16: walrus rejects f32r matmuls with a quadrant-tiled
    # (partition-offset) output, and the SREP r-sum writes pa[64:128]
    srp_d = nc.dram_tensor("srp", [64, 64], BF16, kind="ExternalInput")
    s2c3b_d = nc.dram_tensor("s2c3b", [zp, D], F32R, kind="ExternalInput")

    x_ap = x_d.ap()
    out_ap = out_d.ap()

    with tile.TileContext(nc) as tc:
        with (
            tc.tile_pool(name="consts", bufs=1) as cpool,
            tc.tile_pool(name="xt",
                         bufs=int(os.environ.get("K_XT", "3"))) as pool_xt,
            tc.tile_pool(name="zs",
                         bufs=int(os.environ.get("K_ZS", "8"))) as pool_zs,
            tc.tile_pool(name="stg",
                         bufs=int(os.environ.get("K_STG", "4"))) as pool_stg,
            tc.tile_pool(name="outsb",
                         bufs=int(os.environ.get("K_OUT", "4"))) as pool_out,
            tc.tile_pool(name="ps_a", bufs=int(os.environ.get("K_PA", "2")),
                         space="PSUM") as ps_a,
            tc.tile_pool(name="ps_b", bufs=int(os.environ.get("K_PB", "2")),
                         space="PSUM") as ps_b,
            tc.tile_pool(name="ps_f",
                         bufs=int(os.environ.get("K_PF", "4")),
                         space="PSUM") as ps_f,
        ):
            cp_s = cpool.tile([128, NCH, 384], F8, tag="cp")
            cbh_s = cp_s[:, :, 0:128]
            cbl_s = cp_s[:, :, 128:256]
            cah_s = cp_s[:, :, 256:320]
            cal_s = cp_s[:, :, 320:384]
            sm_s = cpool.tile([64, 192], F32R, tag="sm")
            mb_s = sm_s[:, 0:128]
            ma_s = sm_s[:, 128:192]
            srep_s = cpool.tile([64, 64], BF16, tag="srp")
            s2c3b_s = cpool.tile([zp, D], F32R, tag="s2c3b")

            # ---- PE p-state warm-up: keep TensorE continuously busy while
            # the first DMAs stream in, so real matmuls start at full clock.
            if WARM:
                warm_s = cpool.tile([128, 512], F16, tag="warm")
                nc.gpsimd.memset(warm_s[:], 0.0)
                for w in range(WARM):
                    pw_ = ps_f.tile([128, 512], F32, tag="pf", name="warmpf")
                    nc.tensor.matmul(pw_[:], warm_s[:, 0:128], warm_s[:],
                                     start=True, stop=True)

            def load_x(st, split):
                tb, t0 = st["tb"], st["t0"]
                if split == -1:
                    # lo half into the tile created by the earlier hi load
                    xt = st["xt_tile"]
                    nc.sync.dma_start(xt[:, 1, :, :],
                                      x_ap[:, 1, :, t0:t0 + tb])
                    return
                xt = pool_xt.tile([128, 2, NCH, tb], F8, tag="xt", name="xt")
                if split == 2:
                    # block 0: hi half in pieces so the first DR pass starts
                    # as soon as cp + the first piece landed
                    q = NCH // int(os.environ.get("K_X0SPL", "2"))
                    for j in range(0, NCH, q):
                        nc.sync.dma_start(xt[:, 0, j:j + q, :],
                                          x_ap[:, 0, j:j + q, t0:t0 + tb])
                elif split == 1:
                    nc.sync.dma_start(xt[:, 0, :, :],
                                      x_ap[:, 0, :, t0:t0 + tb])
                else:
                    nc.sync.dma_start(xt[:], x_ap[:, :, :, t0:t0 + tb])
                st["xh"] = xt[:, 0]
                st["xl"] = xt[:, 1]
                st["xt_tile"] = xt

            def hsl(st, u, depth=1):
                q = st["tb"] // (st["H"] if depth == 1 else st["H2"])
                return slice(u * q, (u + 1) * q)

            def dr_passes(st, dst, ch_s, cl_s):
                k, klast = 0, 3 * (NCH // 2) - 1
                for ct, xk in ((ch_s, "xh"), (cl_s, "xh"), (ch_s, "xl")):
                    for jj in range(NCH // 2):
                        js = slice(2 * jj, 2 * jj + 2)
                        nc.tensor.matmul(dst, ct[:, js, :],
                                         st[xk][:, js, :],
                                         start=(k == 0), stop=(k == klast),
                                         perf_mode=DR)
                        k += 1

            def emit_pb(st, depth, u=0):
                """[w0rep | y2] (depth 1: fp8 DR from x; depth 2: MB from
                z2) into one PSUM bank."""
                tb = st["tb"]
                if depth == 1 or u == 0:
                    st[f"pb_{depth}"] = ps_b.tile([128, tb], F32, tag="pb",
                                                  name=f"pb{depth}")
                pb = st[f"pb_{depth}"]
                if depth == 1:
                    dr_passes(st, pb[:], cbh_s, cbl_s)
                else:
                    sl = hsl(st, u, 2)
                    nc.tensor.matmul(pb[:, sl], mb_s, st["z2_1"][:, sl],
                                     start=True, stop=True)

            def emit_pa(st, depth, u=0):
                """y1 (depth 1: fp8 DR; depth 2: MA from z2) into the bottom
                half of a second bank; SREP later fills its top half."""
                tb = st["tb"]
                if depth == 1 or u == 0:
                    st[f"pa_{depth}"] = ps_a.tile([128, tb], F32, tag="pa",
                                                  name=f"pa{depth}")
                pa = st[f"pa_{depth}"]
                if depth == 1:
                    dr_passes(st, pa[0:64, :], cah_s, cal_s)
                else:
                    sl = hsl(st, u, 2)
                    nc.tensor.matmul(pa[0:64, sl], ma_s, st["z2_1"][:, sl],
                                     start=True, stop=True)

            def stage_b(st, depth, eng=0, u=0):
                """ONE copy stages the whole [w0rep | y2] bank to SBUF
                (per half u for depth 2)."""
                tb = st["tb"]
                if u == 0:
                    st[f"stg_{depth}"] = pool_stg.tile(
                        [128, tb], F32R, tag=f"stg{depth}",
                        name=f"stg{depth}")
                stg = st[f"stg_{depth}"]
                sl = hsl(st, u, depth)
                if eng == 0:
                    nc.scalar.copy(stg[:, sl], st[f"pb_{depth}"][:, sl])
                else:
                    nc.vector.tensor_copy(stg[:, sl],
                                          st[f"pb_{depth}"][:, sl])

            def chain_z1(st, depth, u=0, zspl=1):
                """z1 = y1[PSUM] * w0s[SBUF] (optionally in pieces)."""
                tb = st["tb"]
                pa = st[f"pa_{depth}"]
                if u == 0:
                    st[f"z1_{depth}"] = pool_zs.tile(
                        [64, tb], BF16, tag=f"z1{depth}", name=f"z1d{depth}")
                z1 = st[f"z1_{depth}"]
                sl = hsl(st, u, depth)
                n = sl.stop - sl.start
                for v in range(zspl):
                    vs = slice(sl.start + v * n // zspl,
                               sl.start + (v + 1) * n // zspl)
                    nc.vector.tensor_mul(z1[:, vs], pa[0:64, vs],
                                         st[f"stg_{depth}"][0:64, vs])

            def chain_srep(st, depth, u=0, zspl=1):
                """r-sum on TensorE into the top half of the PA bank."""
                pa = st[f"pa_{depth}"]
                sl = hsl(st, u, depth)
                n = sl.stop - sl.start
                for v in range(zspl):
                    vs = slice(sl.start + v * n // zspl,
                               sl.start + (v + 1) * n // zspl)
                    nc.tensor.matmul(pa[64:128, vs], srep_s,
                                     st[f"z1_{depth}"][:, vs],
                                     start=True, stop=True)

            def chain_z2(st, depth, u=0, zspl=1):
                """z2 = pw[PSUM] * y2s[SBUF].  zspl>1 emits the multiply
                in token-tile pieces so downstream finals can start on the
                first piece (used for the last block's z2b)."""
                tb = st["tb"]
                pa = st[f"pa_{depth}"]
                pp = zp if depth == 2 else 64
                if u == 0:
                    st[f"z2_{depth}"] = pool_zs.tile(
                        [pp, tb], F32R, tag=f"z2{depth}", name=f"z2d{depth}")
                z2 = st[f"z2_{depth}"]
                sl = hsl(st, u, depth)
                n = sl.stop - sl.start
                zw = os.environ.get("K_ZW", "128/384")
                ws = ([int(w) for w in zw.split("/")]
                      if zspl > 1 and zw else [])
                if ws and sum(ws) == n:
                    # explicit piece widths: a small tile-aligned first
                    # piece lets the first finals gate earliest
                    bnds = [0]
                    for w in ws:
                        bnds.append(bnds[-1] + w)
                else:
                    bnds = [v * n // zspl for v in range(zspl + 1)]
                for v in range(len(bnds) - 1):
                    vs = slice(sl.start + bnds[v], sl.start + bnds[v + 1])
                    nc.vector.tensor_mul(z2[0:64, vs], pa[64:128, vs],
                                         st[f"stg_{depth}"][64:128, vs])
                if depth == 2 and with_bias and u == st["H2"] - 1:
                    nc.vector.memset(z2[64:65, :], 1.0)

            BIGPF = int(os.environ.get("K_BIGPF", "0"))

            def final_one(st, k, ceng, tail=False):
                """One final matmul [128, 512] per half-tile; bf16 copy
                (rotating engine) per half-tile, or per tile spanning two
                PSUM banks when BIGPF; out DMA per OUTSPLIT group."""
                ntile = st["ntile"]
                i, h = k // 2, k % 2
                if k == 0:
                    st["osb"] = pool_out.tile([128, ntile, D], BF16,
                                              tag="outsb", name="osb")
                z2 = st["z2_2"]
                if BIGPF:
                    if h == 0:
                        st["pf"] = ps_f.tile([128, D], F32, tag="pf",
                                             name="pf")
                    pf = st["pf"][:, h * 512:(h + 1) * 512]
                elif tail and int(os.environ.get("K_TWID", "1")):
                    # ps_a/ps_b are idle at the tail: widen the pf ring
                    tp = (ps_f, ps_a, ps_b)[k % 3]
                    tag = {id(ps_f): "pf", id(ps_a): "pa",
                           id(ps_b): "pb"}[id(tp)]
                    pf = tp.tile([128, 512], F32, tag=tag, name="pf")
                else:
                    pf = ps_f.tile([128, 512], F32, tag="pf", name="pf")
                nc.tensor.matmul(pf[:], z2[:, i * 128:(i + 1) * 128],
                                 s2c3b_s[:, h * 512:(h + 1) * 512],
                                 start=True, stop=True)
                if BIGPF:
                    if h == 0:
                        return
                    src_, dst = st["pf"][:], st["osb"][:, i, :]
                else:
                    src_ = pf[:]
                    dst = st["osb"][:, i, h * 512:(h + 1) * 512]
                if ceng == 0:
                    nc.scalar.copy(dst, src_)
                else:
                    nc.vector.tensor_copy(dst, src_)
                # out DMA per OUTSPLIT group of tiles
                g0 = st["g0"]
                if OUTSPLIT > ntile:
                    nc.sync.dma_start(
                        out_ap[:, g0 + i:g0 + i + 1, h * 512:(h + 1) * 512],
                        st["osb"][:, i:i + 1, h * 512:(h + 1) * 512])
                    return
                osp = min(OUTSPLIT, ntile)
                if h == 1 and (i + 1) % (ntile // osp) == 0:
                    i0g = i + 1 - ntile // osp
                    oq = int(os.environ.get("K_OQ", "0"))
                    deng = (nc.sync, nc.scalar, nc.gpsimd, nc.vector)[oq]
                    if tail and i % 2 == 1 and int(os.environ.get("K_TDQ",
                                                                  "0")):
                        deng = nc.scalar
                    deng.dma_start(out_ap[:, g0 + i0g:g0 + i + 1, :],
                                   st["osb"][:, i0g:i + 1, :])

            # ---- software-pipelined emission --------------------------------
            HB = [int(v) for v in
                  os.environ.get("K_HB", ",".join(["1"] * NBLK)).split(",")]
            H2B = [int(v) for v in
                   os.environ.get("K_H2B",
                                  ",".join(["1"] * NBLK)).split(",")]
            assert len(HB) == NBLK and len(H2B) == NBLK
            state = []
            t0 = 0
            for b, tb in enumerate(BS):
                state.append({"tb": tb, "t0": t0, "g0": t0 // 128,
                              "ntile": tb // 128, "nf": tb // 64,
                              "H": HB[b], "H2": H2B[b]})
                t0 += tb
            # start order: weights, block-0 hi (in pieces; the first 2/3 of
            # each DR pass-group only needs xh), block-0 lo, then block 1
            nc.sync.dma_start(cp_s[:], cp_d.ap()[:])
            load_x(state[0], split=2)
            if int(os.environ.get("K_H1ST", "0")) and NBLK > 1:
                load_x(state[1], split=1)
                load_x(state[0], split=-1)
                nc.sync.dma_start(sm_s[:], sm_d.ap()[:])
                load_x(state[1], split=-1)
            else:
                load_x(state[0], split=-1)
                nc.sync.dma_start(sm_s[:], sm_d.ap()[:])
                nc.sync.dma_start(srep_s[:], srp_d.ap()[:])
                if NBLK > 1:
                    if int(os.environ.get("K_X1SPL", "1")):
                        load_x(state[1], split=1)
                        load_x(state[1], split=-1)
                    else:
                        load_x(state[1], split=0)
            PRELOAD = int(os.environ.get("K_PRELOAD", "0"))
            if PRELOAD:
                for bb in range(2, NBLK):
                    load_x(state[bb], split=0)
            nc.sync.dma_start(s2c3b_s[:], s2c3b_d.ap()[:])

            # copy-engine rotation (0=Act, 2=DVE)
            CROT = [int(c) for c in os.environ.get("K_CROT", "00002020")]
            SE1 = int(os.environ.get("K_SE1", "0"))
            SE2 = int(os.environ.get("K_SE2", "0"))

            for b in range(NBLK + 1):
                st = state[b] if b < NBLK else None
                pv = state[b - 1] if b >= 1 else None

                if st is not None:
                    if b + 2 < NBLK and not PRELOAD:
                        load_x(state[b + 2], split=False)
                    emit_pb(st, 1)                   # PE: 12 DR mm
                if pv is not None:
                    zspl = (int(os.environ.get("K_ZSPL", "1"))
                            if b == NBLK else 1)
                    for u in range(pv["H2"]):
                        chain_srep(pv, 2, u)         # PE (needs z1b(pv))
                        chain_z2(pv, 2, u, zspl)     # DVE -> z2b(pv)
                if st is not None:
                    emit_pa(st, 1)                   # PE: 12 DR mm
                    stage_b(st, 1, SE1)              # Act: [w0|y2] -> SBUF
                    for u in range(st["H"]):
                        chain_z1(st, 1, u)           # DVE (needs pa, stg)

                # interleave positions for block-b stages inside finals(pv)
                NFv = st["nf"] if st is not None else 0
                KSR = int(os.environ.get("K_KSR", "1"))
                KMD = int(os.environ.get("K_KMD", "4"))
                if b == NBLK - 1:
                    KSR = int(os.environ.get("K_KSRL", str(KSR)))
                    KMD = int(os.environ.get("K_KMDL", str(KMD)))
                srp = {}
                if st is not None:
                    hb, h2 = st["H"], st["H2"]
                    step = max(NFv // hb, 1)
                    step2 = max((NFv - KMD) // h2, 1)
                    used = set()
                    for u in range(hb):
                        ks = min(KSR + u * step, NFv - 2 + u)
                        while ks in used:
                            ks += 1
                        srp[ks] = ("srep", u)
                        used.add(ks)
                    for u in range(h2):
                        km = min(KMD + u * step2, NFv - 1 + u)
                        while km in used:
                            km += 1
                        srp[km] = ("mid", u)
                        used.add(km)
                done = set()

                def blk_stage(k):
                    if st is None or k not in srp or k in done:
                        return
                    done.add(k)
                    kind, u = srp[k]
                    if kind == "srep":
                        chain_srep(st, 1, u)         # PE: srep1
                        chain_z2(st, 1, u)           # DVE -> z2(st)
                    else:
                        emit_pb(st, 2, u)            # PE: MB
                        emit_pa(st, 2, u)            # PE: MA
                        stage_b(st, 2, SE2, u)       # Act
                        z1spl = (int(os.environ.get("K_Z1SPL", "1"))
                                 if b == NBLK - 1 else 1)
                        chain_z1(st, 2, u, z1spl)    # DVE -> z1b(st)

                if pv is not None:
                    tail = b == NBLK
                    for k in range(pv["nf"]):
                        blk_stage(k)
                        ce = (2, 0)[(k + int(os.environ.get("K_TCE", "0")))
                                    % 2] if tail else CROT[k % len(CROT)]
                        final_one(pv, k, ce, tail=tail)
                for k in sorted(srp):
                    if k not in done:
                        blk_stage(k)

    nc.compile()
    return nc


def _constants(core0, core1, core2, core3, bias):
    core0 = np.asarray(core0, np.float64)
    core1 = np.asarray(core1, np.float64)
    core2 = np.asarray(core2, np.float64)
    core3 = np.asarray(core3, np.float64)
    bias = np.asarray(bias, np.float64)

    # k index = r*8 + R  (prev rank r, next rank R)
    C0REP = np.repeat(core0[0], R, axis=1)                 # (D, 64)
    C1 = core1.transpose(1, 0, 2).reshape(D, 64)           # y1
    C2 = core2.transpose(1, 0, 2).reshape(D, 64)           # y2
    CB = np.concatenate([C0REP, C2], axis=1)               # (D, 128)
    SREP = np.kron(np.ones((R, 1)),
                   np.kron(np.eye(R), np.ones((1, R))))    # (64,64)
    S2 = np.tile(np.eye(R), (R, 1))                        # (64,8)
    C3S = np.tile(core3[:, :, 0], (R, 1))                  # (64,D)
    # host-folded depth boundary (depth-1 state carries WS per
    # x-contraction => z2 ~ WS^3; fold 1/WS^3 here)
    MB = C3S @ CB / WS**3                                  # (64, 128)
    MA = C3S @ C1 / WS**3                                  # (64, 64)
    S2C3B = S2 @ core3[:, :, 0]                            # (64,D)

    import ml_dtypes
    F8NP = ml_dtypes.float8_e4m3

    def chunk_major3(a, po):
        # (D, po) -> (128, NCH, po): partition = d%128, chunk = d//128
        return np.ascontiguousarray(
            a.reshape(NCH, 128, po).transpose(1, 0, 2))

    def hi_lo8(a):
        hi = np.asarray(a, np.float64).astype(F8NP)
        lo = (np.asarray(a, np.float64) - hi.astype(np.float64)).astype(F8NP)
        return hi, lo

    with_bias = bool(np.any(bias))
    cbh, cbl = hi_lo8(chunk_major3(CB * WS, 128))
    cah, cal = hi_lo8(chunk_major3(C1 * WS, 64))
    cp = np.concatenate([cbh, cbl, cah, cal], axis=2)      # (128, NCH, 384)
    sm = np.concatenate([MB, MA], axis=1)

    if with_bias:
        s2c3b = np.concatenate([S2C3B, bias[None, :]], axis=0)
    else:
        s2c3b = S2C3B
    s2c3b = np.ascontiguousarray(s2c3b).astype(np.float32)

    consts = {
        "cp": np.ascontiguousarray(cp),
        "sm": np.ascontiguousarray(sm).astype(np.float32),
        "srp": np.ascontiguousarray(SREP).astype(ml_dtypes.bfloat16),
        "s2c3b": s2c3b,
    }
    return consts, with_bias


_NC_CACHE = {}


def _get_program(with_bias=False):
    if with_bias not in _NC_CACHE:
        _NC_CACHE[with_bias] = _build_program(with_bias)
    return _NC_CACHE[with_bias]


def run(x, core0, core1, core2, core3, bias, trace=False, **spmd_kwargs):
    import ml_dtypes
    F8NP = ml_dtypes.float8_e4m3
    consts, with_bias = _constants(core0, core1, core2, core3, bias)
    nc = _get_program(with_bias)
    xf = np.asarray(x, np.float64).reshape(T_TOTAL, D)
    xh = xf.astype(F8NP)
    xl = (xf - xh.astype(np.float64)).astype(F8NP)
    in_maps = []
    for c in range(N_CORES):
        m = dict(consts)
        sl = slice(c * T_CORE, (c + 1) * T_CORE)
        # d-major [128, 2, NCH, T_CORE]: partition p=d%128, chunk j=d//128,
        # dim1 = hi/lo
        xd = np.stack(
            [arr[sl].T.reshape(NCH, 128, T_CORE).transpose(1, 0, 2)
             for arr in (xh, xl)], axis=1)
        m["x8"] = np.ascontiguousarray(xd)
        in_maps.append(m)
    res = bass_utils.run_bass_kernel_spmd(
        nc, in_maps, core_ids=list(range(N_CORES)), trace=trace, **spmd_kwargs)
    outs = []
    for c in range(N_CORES):
        oc = np.asarray(res.results[c]["out"])    # [128, TTILES, D] bf16
        # token t = g*128 + p
        outs.append(oc.transpose(1, 0, 2).reshape(T_CORE, D))
    out = np.concatenate(outs, axis=0).astype(np.float32)
    return out.reshape(B, N, D), res


def kernel(x, core0, core1, core2, core3, bias):
    out, _ = run(x, core0, core1, core2, core3, bias)
    return out
